# revision 1
# baseline (speedup 1.0000x reference)
"""Trainium2 Bass kernel for the parameterized-quantum-circuit policy network.

Math: the circuit is psi = V5 E4 V4 ... E0 V0 e0 where V_l are x-independent
1024x1024 unitaries (single-qubit rotations + CZ ring, all built from theta)
and E_l(x) = tensor-prod of Rx(lam*x). Using Rx = H Rz H (H = Hadamard^{ox10}),
E_l = H D_l(x) H with D_l diagonal. Folding the H's into the V's:

    psi = W5 D4 W4 D3 W3 D2 W2 D1 W1 (D0 * psi1)

with W_l = H V_l H (l=1..4), W5 = V5 H, psi1 = first column of H V0, and
D_l[b,k] = exp(-i * phi), phi = sum_q (1-2 bits[k,q]) * lam[l,q] * x[b,q] / 2.

Device work per core (batch-sharded 2048 -> 8 x 256, state [1024, 256] with
dim on partitions): 5 complex 1024x1024 matmuls (Karatsuba: 3 fp32 real
matmuls each), diagonal phase multiplies (phi via K=10 matmul, sin/cos on ACT
with rint range reduction), readout sum(|psi|^2 * Zsign) via M=1 reduce
matmuls, sigmoid for the 2-way softmax. All theta/lam/w-derived tables are
host-precomputed; all x-dependent compute runs on device.
"""

import sys

sys.path.insert(0, "/opt/trn_rl_repo")

import numpy as np
import concourse.bass as bass
import concourse.mybir as mybir
import concourse.tile as tile
from concourse.bass_utils import run_bass_kernel_spmd

F32 = mybir.dt.float32
F32R = mybir.dt.float32r
AF = mybir.ActivationFunctionType
ALU = mybir.AluOpType

NQ = 10
DIM = 1024
L = 5
B = 2048
NC = 8
BC = B // NC  # 256 batch per core
KT = DIM // 128  # 8 k tiles
BETA = 1.0

PI = float(np.pi)
MAGIC = float(1.5 * 2**23)
INV2PI = float(1.0 / (2.0 * np.pi))
TWOPI = float(2.0 * np.pi)


# ---------------------------------------------------------------- host math
_bits = (np.arange(DIM)[:, None] >> (NQ - 1 - np.arange(NQ))) & 1
_SIGNS = (1.0 - 2.0 * _bits).astype(np.float64)
_cz = np.ones(DIM)
for _i in range(NQ):
    _cz *= 1.0 - 2.0 * (_bits[:, _i] * _bits[:, (_i + 1) % NQ])
_ZSIGN = (1.0 - 2.0 * (_bits.sum(1) % 2)).astype(np.float64)


def _rx(t):
    c, s = np.cos(0.5 * t), np.sin(0.5 * t)
    return np.array([[c, -1j * s], [-1j * s, c]])


def _ry(t):
    c, s = np.cos(0.5 * t), np.sin(0.5 * t)
    return np.array([[c, -s], [s, c]])


def _rz(t):
    e = np.exp(-0.5j * t)
    return np.array([[e, 0.0], [0.0, np.conj(e)]])


def _build_weights(theta, lam):
    th = np.asarray(theta, np.float64).reshape(L + 1, NQ, 3)
    lm = np.asarray(lam, np.float64).reshape(L, NQ)
    H1 = np.array([[1.0, 1.0], [1.0, -1.0]]) / np.sqrt(2.0)
    H = np.array([[1.0]])
    for _ in range(NQ):
        H = np.kron(H, H1)
    V = []
    for l in range(L + 1):
        U = np.array([[1.0]], dtype=np.complex128)
        for q in range(NQ):
            U = np.kron(U, _rz(th[l, q, 2]) @ _ry(th[l, q, 1]) @ _rx(th[l, q, 0]))
        V.append(_cz[:, None] * U)
    psi1 = (H @ V[0])[:, 0]
    W = [H @ V[l] @ H for l in range(1, L)] + [V[L] @ H]
    A = np.empty((L, NQ, DIM))
    for l in range(L):
        A[l] = (_SIGNS * (lm[l] / 2.0)).T
    return W, psi1, A


# ---------------------------------------------------------------- device IR
def _legalize_single_wait(nc):
    """This walrus build accepts only one sync-wait per instruction: hoist
    extra waits onto injected single-wait EventSemaphore carriers."""
    n_fix = 0
    for f in nc.m.functions:
        for bb in f.blocks:
            insts = bb.instructions
            new = []
            for ins in insts:
                si = ins.sync_info
                if si is not None and len(si.on_wait) > 1:
                    for w in si.on_wait[:-1]:
                        n_fix += 1
                        ev = mybir.InstEventSemaphore(
                            name=f"waitfix_{ins.name}_{n_fix}", ins=[], outs=[]
                        )
                        ev.engine = ins.engine
                        ev.sync_info = mybir.SyncInfo(on_wait=[w], on_update=[])
                        new.append(ev)
                    ins.sync_info = mybir.SyncInfo(
                        on_wait=[si.on_wait[-1]], on_update=si.on_update
                    )
                new.append(ins)
            insts[:] = new
    return n_fix


def _build_nc(mm_f32r=False, debug=False, repeat=1, internal_weights=False):
    nc = bass.Bass()
    wdt = F32R if mm_f32r else F32
    wkind = "Internal" if internal_weights else "ExternalInput"

    xt_d = nc.dram_tensor("xt", [NQ, BC], F32, kind="ExternalInput")
    at_d = nc.dram_tensor("at", [NQ, L, DIM], F32, kind="ExternalInput")
    psire_d = nc.dram_tensor("psire", [128, KT], F32, kind="ExternalInput")
    psiim_d = nc.dram_tensor("psiim", [128, KT], F32, kind="ExternalInput")
    zs_d = nc.dram_tensor("zs", [128, KT], wdt, kind="ExternalInput")
    wsc_d = nc.dram_tensor("wsc", [1, 1], F32, kind="ExternalInput")
    wall_d = {}
    for l in range(1, L + 1):
        wall_d[l] = nc.dram_tensor(
            f"wall{l}", [KT, 4, 128, 512], wdt, kind=wkind
        )
    probs_d = nc.dram_tensor("probs", [2, BC], F32, kind="ExternalOutput")
    if debug:
        dbga_d = nc.dram_tensor(
            "dbga", [L + 1, 128, KT, BC], F32, kind="ExternalOutput"
        )
        dbgb_d = nc.dram_tensor(
            "dbgb", [L + 1, 128, KT, BC], F32, kind="ExternalOutput"
        )

    with tile.TileContext(nc) as tc:
        with (
            tc.tile_pool(name="consts", bufs=1) as cpool,
            tc.tile_pool(name="state", bufs=2) as spool,
            tc.tile_pool(name="wts", bufs=4) as wpool,
            tc.tile_pool(name="trig", bufs=2) as tpool,
            tc.tile_pool(name="scr", bufs=6) as upool,
            tc.tile_pool(name="outp", bufs=1) as opool,
            tc.tile_pool(name="psum", bufs=1, space="PSUM") as ppool,
        ):
            # ---- constants
            xt_t = cpool.tile([NQ, BC], F32)
            nc.sync.dma_start(xt_t[:], xt_d[:])
            at_t = cpool.tile([NQ, L, DIM], F32)
            nc.sync.dma_start(at_t[:], at_d[:])
            psire_t = cpool.tile([128, KT], F32)
            nc.sync.dma_start(psire_t[:], psire_d[:])
            psiim_t = cpool.tile([128, KT], F32)
            nc.sync.dma_start(psiim_t[:], psiim_d[:])
            zs_t = cpool.tile([128, KT], wdt)
            nc.sync.dma_start(zs_t[:], zs_d[:])
            wsc_t = cpool.tile([1, 1], F32)
            nc.sync.dma_start(wsc_t[:], wsc_d[:])
            zbias = cpool.tile([128, 1], F32)
            nc.vector.memset(zbias[:], 0.0)
            hpi = cpool.tile([128, 1], F32)
            nc.vector.memset(hpi[:], PI / 2)
            zb1 = cpool.tile([1, 1], F32)
            nc.vector.memset(zb1[:], 0.0)

            def compute_phase(l, ct_t, st_t):
                """ct/st [128, KT, BC] <- cos/sin(2pi * phi') per k-tile, where
                phi' = phi/2pi comes out of the PE (A tables pre-divided by
                2pi on the host); rint range reduction, Sin scale=2pi."""
                for t in range(KT):
                    phi_p = ppool.tile([128, BC], F32, tag="scratch", name="phi_p")
                    nc.tensor.matmul(
                        phi_p[:],
                        at_t[:, l, 128 * t : 128 * (t + 1)],
                        xt_t[:],
                        start=True,
                        stop=True,
                    )
                    n1 = upool.tile([128, BC], F32, tag="rn", name="n1")
                    nc.vector.tensor_scalar(
                        n1[:], phi_p[:], MAGIC, -MAGIC, ALU.add, ALU.add
                    )
                    fr = upool.tile([128, BC], F32, tag="rf", name="fr")
                    nc.vector.scalar_tensor_tensor(
                        fr[:], n1[:], -1.0, phi_p[:], ALU.mult, ALU.add
                    )
                    nc.scalar.activation(
                        st_t[:, t, :], fr[:], AF.Sin, bias=zbias[:], scale=TWOPI
                    )
                    n2 = upool.tile([128, BC], F32, tag="rn", name="n2")
                    nc.vector.tensor_scalar(
                        n2[:], phi_p[:], 0.25, MAGIC, ALU.add, ALU.add
                    )
                    nc.vector.tensor_scalar_add(n2[:], n2[:], -MAGIC)
                    fr2 = upool.tile([128, BC], F32, tag="rf", name="fr2")
                    nc.vector.scalar_tensor_tensor(
                        fr2[:], n2[:], -1.0, phi_p[:], ALU.mult, ALU.add
                    )
                    nc.scalar.activation(
                        ct_t[:, t, :], fr2[:], AF.Sin, bias=hpi[:], scale=TWOPI
                    )

            def emit_round(dump_debug):
                # ---- init: state = D_0 * psi1
                ct0 = tpool.tile([128, KT, BC], F32, tag="ct", name="ct0")
                st0 = tpool.tile([128, KT, BC], F32, tag="st", name="st0")
                compute_phase(0, ct0, st0)
                a_t = spool.tile([128, KT, BC], wdt, tag="sa", name="a0")
                b_t = spool.tile([128, KT, BC], wdt, tag="sb", name="b0")
                for t in range(KT):
                    eng = nc.vector
                    u1 = upool.tile([128, BC], F32, tag="u1", name="iu1")
                    eng.tensor_scalar_mul(u1[:], ct0[:, t, :], psire_t[:, t : t + 1])
                    eng.scalar_tensor_tensor(
                        a_t[:, t, :],
                        st0[:, t, :],
                        psiim_t[:, t : t + 1],
                        u1[:],
                        ALU.mult,
                        ALU.add,
                    )
                    u2 = upool.tile([128, BC], F32, tag="u2", name="iu2")
                    eng.tensor_scalar_mul(u2[:], st0[:, t, :], psire_t[:, t : t + 1])
                    eng.scalar_tensor_tensor(
                        b_t[:, t, :],
                        ct0[:, t, :],
                        psiim_t[:, t : t + 1],
                        u2[:],
                        ALU.mult,
                        ALU.subtract,
                    )
                if dump_debug:
                    nc.sync.dma_start(dbga_d[0], a_t[:])
                    nc.sync.dma_start(dbgb_d[0], b_t[:])

                # ---- layers
                for l in range(1, L + 1):
                    if l < L:
                        ctl = tpool.tile([128, KT, BC], F32, tag="ct", name="ctl")
                        stl = tpool.tile([128, KT, BC], F32, tag="st", name="stl")
                        compute_phase(l, ctl, stl)
                    a2_t = spool.tile([128, KT, BC], wdt, tag="sa", name="a2")
                    b2_t = spool.tile([128, KT, BC], wdt, tag="sb", name="b2")
                    rhs_a, rhs_b = a_t, b_t
                    for pass_ in range(4):
                        # one PSUM bank per accumulation group: start=True
                        # zeroes the whole bank, so groups must not share
                        banks = {
                            p: [
                                ppool.tile(
                                    [128, BC], F32, tag=f"m{p}a", name=f"m{p}a"
                                ),
                                ppool.tile(
                                    [128, BC], F32, tag=f"m{p}b", name=f"m{p}b"
                                ),
                            ]
                            for p in "cdr"
                        }
                        for k in range(KT):
                            wt = wpool.tile(
                                [128, 512], wdt, tag="wall", name="wallt", bufs=12
                            )
                            dma_eng = (nc.sync, nc.sync, nc.sync, nc.scalar)[k % 4]
                            dma_eng.dma_start(wt[:], wall_d[l][k, pass_])
                            for mi in range(2):
                                wc_sl = wt[:, 128 * mi : 128 * mi + 128]
                                wn_sl = wt[:, 256 + 128 * mi : 256 + 128 * mi + 128]
                                # re = c@a + d'@b (d' = -d, one 16-mm group)
                                nc.tensor.matmul(
                                    banks["c"][mi][:],
                                    wc_sl,
                                    rhs_a[:, k, :],
                                    start=(k == 0),
                                    stop=False,
                                    skip_group_check=True,
                                )
                                nc.tensor.matmul(
                                    banks["c"][mi][:],
                                    wn_sl,
                                    rhs_b[:, k, :],
                                    start=False,
                                    stop=(k == KT - 1),
                                    skip_group_check=True,
                                )
                                # im = c@b - d'@a
                                nc.tensor.matmul(
                                    banks["d"][mi][:],
                                    wc_sl,
                                    rhs_b[:, k, :],
                                    start=(k == 0),
                                    stop=(k == KT - 1),
                                    skip_group_check=True,
                                )
                                nc.tensor.matmul(
                                    banks["r"][mi][:],
                                    wn_sl,
                                    rhs_a[:, k, :],
                                    start=(k == 0),
                                    stop=(k == KT - 1),
                                    skip_group_check=True,
                                )
                        for mi in range(2):
                            mg = 2 * pass_ + mi
                            m_re = banks["c"][mi][:]
                            m_cb = banks["d"][mi][:]
                            m_da = banks["r"][mi][:]
                            nc.vector.tensor_copy(a2_t[:, mg, :], m_re)
                            u5 = upool.tile([128, BC], F32, tag="u3", name="u5")
                            nc.scalar.copy(u5[:], m_da)
                            nc.vector.tensor_sub(b2_t[:, mg, :], m_cb, u5[:])
                    if l < L:
                        stn = tpool.tile([128, KT, BC], F32, tag="stn", name="stn")
                        nc.vector.tensor_scalar_mul(stn[:], stl[:], -1.0)
                        for t in range(KT):
                            eng = nc.gpsimd
                            u1 = upool.tile([128, BC], F32, tag="u1", name="du1")
                            u2 = upool.tile([128, BC], F32, tag="u2", name="du2")
                            u3 = upool.tile([128, BC], F32, tag="u3", name="du3")
                            u4 = upool.tile([128, BC], F32, tag="u4", name="du4")
                            eng.tensor_mul(u1[:], ctl[:, t, :], a2_t[:, t, :])
                            eng.tensor_mul(u2[:], stl[:, t, :], b2_t[:, t, :])
                            eng.tensor_mul(u3[:], ctl[:, t, :], b2_t[:, t, :])
                            eng.tensor_mul(u4[:], stn[:, t, :], a2_t[:, t, :])
                            eng.tensor_add(a2_t[:, t, :], u1[:], u2[:])
                            eng.tensor_add(b2_t[:, t, :], u3[:], u4[:])
                    if dump_debug:
                        nc.sync.dma_start(dbga_d[l], a2_t[:])
                        nc.sync.dma_start(dbgb_d[l], b2_t[:])
                    a_t, b_t = a2_t, b2_t

                # ---- readout
                ez_p = ppool.tile([1, BC], F32, tag="scratch", name="ez")
                for t in range(KT):
                    sq1 = upool.tile([128, BC], wdt, tag="u1", name="sq1")
                    sq2 = upool.tile([128, BC], wdt, tag="u2", name="sq2")
                    nc.gpsimd.tensor_mul(sq1[:], a_t[:, t, :], a_t[:, t, :])
                    nc.gpsimd.tensor_mul(sq2[:], b_t[:, t, :], b_t[:, t, :])
                    nc.tensor.matmul(
                        ez_p[:],
                        zs_t[:, t : t + 1],
                        sq1[:],
                        start=(t == 0),
                        stop=False,
                        skip_group_check=True,
                    )
                    nc.tensor.matmul(
                        ez_p[:],
                        zs_t[:, t : t + 1],
                        sq2[:],
                        start=False,
                        stop=(t == KT - 1),
                        skip_group_check=True,
                    )
                p0 = opool.tile([1, BC], F32, tag="p0", name="p0")
                nc.scalar.activation(
                    p0[:], ez_p[:], AF.Sigmoid, bias=zb1[:], scale=wsc_t[:, :]
                )
                p1 = opool.tile([1, BC], F32, tag="p1", name="p1")
                nc.vector.tensor_scalar(p1[:], p0[:], -1.0, 1.0, ALU.mult, ALU.add)
                nc.sync.dma_start(probs_d[0:1, :], p0[:])
                nc.sync.dma_start(probs_d[1:2, :], p1[:])

            for _rep in range(repeat):
                emit_round(debug and _rep == 0)

    nc.finalize()
    _legalize_single_wait(nc)
    return nc


_NC_CACHE = {}


def _get_nc(mm_f32r, debug=False, repeat=1, internal_weights=False):
    key = (bool(mm_f32r), bool(debug), int(repeat), bool(internal_weights))
    if key not in _NC_CACHE:
        _NC_CACHE[key] = _build_nc(
            mm_f32r=key[0], debug=key[1], repeat=key[2], internal_weights=key[3]
        )
    return _NC_CACHE[key]


def _make_in_maps(x, theta, lam, w):
    W, psi1, A = _build_weights(theta, lam)
    shared = {
        "at": np.ascontiguousarray(A.transpose(1, 0, 2) / (2.0 * np.pi)).astype(
            np.float32
        ),
        "psire": np.ascontiguousarray(psi1.real.reshape(KT, 128).T).astype(
            np.float32
        ),
        "psiim": np.ascontiguousarray(psi1.imag.reshape(KT, 128).T).astype(
            np.float32
        ),
        "zs": np.ascontiguousarray(_ZSIGN.reshape(KT, 128).T).astype(np.float32),
        "wsc": np.array([[BETA * (float(w[0, 0]) - float(w[0, 1]))]], np.float32),
    }
    for l in range(1, L + 1):
        WT = W[l - 1].T

        def _pack(plane):
            # [1024, 1024] -> [KT, 4pass, 128part, 256cols]
            return plane.reshape(KT, 128, 4, 256).transpose(0, 2, 1, 3)

        c = _pack(WT.real)
        dn = _pack(-WT.imag)  # negated: re = c@a + d'@b in one PSUM group
        shared[f"wall{l}"] = np.ascontiguousarray(
            np.concatenate([c, dn], axis=3)
        ).astype(np.float32)
    x = np.asarray(x, np.float32)
    in_maps = []
    for i in range(NC):
        m = dict(shared)
        m["xt"] = np.ascontiguousarray(x[BC * i : BC * (i + 1)].T).astype(np.float32)
        in_maps.append(m)
    return in_maps


def run(x, theta, lam, w, trace=False, mm_f32r=False, debug=False, repeat=1):
    nc = _get_nc(mm_f32r, debug, repeat)
    in_maps = _make_in_maps(x, theta, lam, w)
    res = run_bass_kernel_spmd(nc, in_maps, list(range(NC)), trace=trace)
    out = np.empty((B, 2), np.float32)
    for i in range(NC):
        out[BC * i : BC * (i + 1)] = res.results[i]["probs"].T
    return out, res


def kernel(x, theta, lam, w):
    out, _ = run(x, theta, lam, w, trace=False, mm_f32r=True)
    return out

